# revision 1
# baseline (speedup 1.0000x reference)
"""Embedding lookup (gather) kernel for Trainium2, 8 NeuronCores.

Reference computes emb[b,s,:] = weight[x[b,s],:] (one-hot matmul in the
reference; semantically a row gather). Implementation: data-parallel over
the B*S = 4096 tokens, 512 tokens per core. Each core gathers its 512 rows
of the [32000, 512] f32 table straight from HBM into SBUF with four
indirect DMAs (the HW DGE consumes exactly one row-offset per SBUF
partition, so 128 rows per instruction), then streams each 128-row chunk
back out to its output slice. Stores run on the HWDGE (sync) ring and
chase the gathers, overlapping SWDGE descriptor generation of later
chunks with data movement of earlier ones.

Token layout per core is j-major: gather j handles tokens j*128..(j+1)*128
with token j*128+p on partition p, so each store writes a fully contiguous
256 KiB block of the output and the host-side unshard is a plain reshape.
"""

import numpy as np

import concourse.bass as bass
from concourse import mybir
from concourse.bass_utils import run_bass_kernel_spmd

B, S = 4, 1024
V, D = 32000, 512
N_CORES = 8
TOK = B * S                      # 4096 total tokens
TPC = TOK // N_CORES             # 512 tokens per core
P = 128                          # SBUF partitions
NCH = TPC // P                   # 4 gathers of 128 rows

_CACHE: dict = {}


def _build() -> bass.Bass:
    nc = bass.Bass()
    idx = nc.dram_tensor("idx", [P, NCH], mybir.dt.int32, kind="ExternalInput")
    w = nc.dram_tensor("weight", [V, D], mybir.dt.float32, kind="ExternalInput")
    out = nc.dram_tensor("out", [TPC, D], mybir.dt.float32, kind="ExternalOutput")
    with (
        nc.Block() as block,
        nc.semaphore("idx_sem") as idx_sem,
        nc.semaphore("g0") as g0,
        nc.semaphore("g1") as g1,
        nc.semaphore("g2") as g2,
        nc.semaphore("g3") as g3,
        nc.semaphore("wu") as wu,
        nc.semaphore("wm") as wm,
        nc.semaphore("s_sem") as s_sem,
        nc.sbuf_tensor("idx_t", [P, NCH], mybir.dt.int32) as idx_t,
        nc.sbuf_tensor("emb", [P, NCH * D], mybir.dt.float32) as emb,
        nc.sbuf_tensor("off0", [P, 1], mybir.dt.int32) as off0,
        nc.sbuf_tensor("scr", [P, 4], mybir.dt.float32) as scr,
    ):
        gsems = [g0, g1, g2, g3]

        @block.sync
        def _(s):
            s.dma_start(out=idx_t[:], in_=idx[:]).then_inc(idx_sem, 16)

        @block.gpsimd
        def _(g):
            # warm the SWDGE ring with a tiny row-0 gather while the idx DMA
            # is still in flight — pays first-instruction overhead off the
            # critical path
            g.memset(off0[:], 0).then_inc(wm, 1)
            g.wait_ge(wm, 1)
            g.indirect_dma_start(
                out=scr[:],
                out_offset=None,
                in_=w[:],
                in_offset=bass.IndirectOffsetOnAxis(ap=off0[:, :1], axis=0),
            ).then_inc(wu, 16)
            g.wait_ge(idx_sem, 16)
            for j in range(NCH):
                g.indirect_dma_start(
                    out=emb[:, j * D : (j + 1) * D],
                    out_offset=None,
                    in_=w[:],
                    in_offset=bass.IndirectOffsetOnAxis(ap=idx_t[:, j : j + 1], axis=0),
                ).then_inc(gsems[j], 16)

        @block.sync
        def _(s):
            for j in range(NCH):
                s.wait_ge(gsems[j], 16)
                s.dma_start(
                    out=out[j * P : (j + 1) * P, :],
                    in_=emb[:, j * D : (j + 1) * D],
                ).then_inc(s_sem, 16)
            # no explicit wait on s_sem: the block-end DRAIN on the sync
            # engine waits for HWDGE queue completion (verified exact on HW)

    return nc


def kernel(x: np.ndarray, weight: np.ndarray) -> np.ndarray:
    x = np.asarray(x)
    weight = np.ascontiguousarray(np.asarray(weight, dtype=np.float32))
    flat = np.ascontiguousarray(x.reshape(-1)).astype(np.int32)

    if "nc" not in _CACHE:
        _CACHE["nc"] = _build()
    nc = _CACHE["nc"]

    in_maps = [
        {
            # idx_t[p, j] = token j*128+p of this core's slice (j-major)
            "idx": np.ascontiguousarray(
                flat[i * TPC : (i + 1) * TPC].reshape(NCH, P).T
            ),
            "weight": weight,
        }
        for i in range(N_CORES)
    ]
    res = run_bass_kernel_spmd(nc, in_maps, list(range(N_CORES)))
    outs = [np.asarray(res.results[i]["out"]) for i in range(N_CORES)]
    return np.concatenate(outs, axis=0).reshape(B, S, D)

